# revision 41
# baseline (speedup 1.0000x reference)
"""GCN layer on 8 trn2 NeuronCores.

out = tanh( (D^-1/2 (adj+I) D^-1/2) @ H @ W.T + b ), N=8192, nin=nout=512.

Associativity + normalization folding: with d = deg^-0.5,
  out = tanh( S''^T @ HsW + b )  where
  S''[k, m] = d_m * (adj + I)[m, k]   (fully-normalized adjacency, on host)
  HsW[k, :] = d_k * (H @ W.T)[k, :]   (W folded into H on host: one small
                                       4.3-GFLOP BLAS gemm)
so the device runs a SINGLE big gemm (8192x1024x512 per core) plus a fused
bias+tanh activation per PSUM bank. No second gemm, no transposes, no
PSUM->SBUF copies.

Mixed-precision contraction: the first K1=56 k-tiles run in fp8-e4m3 with
perf_mode=DoubleRow (2 k-tiles per matmul at the same 216ns issue cadence =
2x PE throughput); the last 8 k-tiles run in bf16. The fp8 operand S'' is
mean-centered per output column (c_m = mean_k S''[k,m]) before quantization
— halving its quantization error — and the mean contribution (rank-1 term
c_m (x) colsum(HsW)) is folded back into the bf16-phase S data on host: a
least-norm w with Hbf^T w = colsum is added as c (x) w, so the ordinary
bf16 contraction restores the mean with zero extra device work. Both
operands are pre-scaled by 512 (values ~0.0156 sit in e4m3's subnormal
range); the bf16 operands carry the same scales so all products share the
2^18 factor, removed by the final activation's scale. Measured end-to-end
rel err 1.753e-2 (L2, bit-stable across runs) vs the 2e-2 gate; max abs
err ~1.4e-3.

Sharding: output rows (and adj rows) split across 8 cores, 1024 rows each.
Output lands transposed ([nout, m] blocks); the host transposes it back.

All SWDGE DMAs drain FIFO on one logical queue, so issue order is arrival
order: HsW slices are interleaved with the S strip chunks in exactly the
order the k-loop consumes them, and the first chunks are small to cut
startup latency. Short dummy matmuls on scratch warm the PE's HAM clock
gate during the initial DMA wait. The last chunk runs bank-major so banks
stop staggered and the tanh+store tail overlaps the remaining matmuls.
"""

import sys

sys.path.insert(0, "/opt/trn_rl_repo")

import numpy as np
import ml_dtypes

from concourse import bass, bacc, tile, mybir
from concourse.bass_utils import run_bass_kernel_spmd

N = 8192
NIN = 512
NOUT = 512
NC = 8
RB = N // NC  # 1024 rows per core
KT = N // 128  # 64 k-tiles
K1 = 56  # k-tiles in fp8 DoubleRow; rest in bf16
KB = KT - K1
CH8 = [2, 2, 4] + [8] * 6  # fp8-phase chunk sizes (sum = K1, all even)
CHB = [8]  # bf16-phase chunk sizes (sum = KB)
SC = 512.0  # per-operand prescale (e4m3 subnormal floor is 2^-6)
F32 = mybir.dt.float32
BF16 = mybir.dt.bfloat16
FP8 = mybir.dt.float8e4
NPBF = ml_dtypes.bfloat16
NP8 = mybir.dt.np(mybir.dt.float8e4)

_CACHED_NC = None


def _build():
    nc = bacc.Bacc(None, target_bir_lowering=False)

    # Per-core inputs (packed layouts, see kernel() glue)
    S8 = nc.dram_tensor("S8", [128, K1, RB], FP8, kind="ExternalInput")
    Sb = nc.dram_tensor("Sb", [128, KB, RB], BF16, kind="ExternalInput")
    HW8 = nc.dram_tensor("HW8", [128, K1, NOUT], FP8, kind="ExternalInput")
    HWB = nc.dram_tensor("HWB", [128, KB, NOUT], BF16, kind="ExternalInput")
    Bt = nc.dram_tensor("Bt", [128, 4], F32, kind="ExternalInput")
    # Output transposed: col block (c*2+mb)*512 holds OutT[c-chunk, mb-half]
    Out = nc.dram_tensor("out", [128, 8 * 512], BF16, kind="ExternalOutput")

    with tile.TileContext(nc) as tc:
        with (
            tc.tile_pool(name="persist", bufs=1) as persist,
            tc.tile_pool(name="strip8", bufs=6) as strip8p,
            tc.tile_pool(name="stripb", bufs=2) as stripbp,
            tc.tile_pool(name="res", bufs=4) as resp,
            tc.tile_pool(name="acc", bufs=2, space=bass.MemorySpace.PSUM) as pacc,
        ):
            hw8_big = persist.tile([128, K1, NOUT], FP8)
            hwb_big = persist.tile([128, KB, NOUT], BF16)
            b_t = persist.tile([128, 4], F32)

            # Both m-halves accumulate across the whole k loop: 8 banks.
            acc0 = pacc.tile([128, 4 * 512], F32, tag="acc")
            acc1 = pacc.tile([128, 4 * 512], F32, tag="acc")
            accs = (acc0, acc1)

            # HAM warm-up (see module docstring).
            scratch = persist.tile([128, 128], BF16)
            nc.vector.memset(scratch[:], 0.0)
            for _ in range(44):
                nc.tensor.matmul(
                    acc0[:, 0:128], scratch[:], scratch[:], start=True, stop=True
                )

            # ---- fp8 DoubleRow phase: kt in [0, K1), 2 k-tiles per matmul
            kt0 = 0
            for ci, cn in enumerate(CH8):
                # First two chunks ride the HWDGE ring (lower first-byte
                # latency, parallel to the SWDGE stream) so the gemm starts
                # sooner while SWDGE ramps on the later chunks.
                deng = nc.sync if ci <= 1 else nc.gpsimd
                hsl = slice(kt0, kt0 + cn)
                deng.dma_start(hw8_big[:, hsl, :], HW8[:, hsl, :])
                strip = strip8p.tile([128, 8, RB], FP8, tag="s8")
                for s0 in range(0, cn, 4):
                    sn = min(4, cn - s0)
                    deng.dma_start(
                        strip[:, s0 : s0 + sn, :], S8[:, kt0 + s0 : kt0 + s0 + sn, :]
                    )
                if ci == 1:
                    nc.gpsimd.dma_start(b_t[:], Bt[:, :])
                for ktl in range(0, cn, 2):
                    kt = kt0 + ktl
                    for c in range(4):
                        for mb in range(2):
                            nc.tensor.matmul(
                                accs[mb][:, c * 512 : (c + 1) * 512],
                                hw8_big[:, ktl + kt0 : ktl + kt0 + 2, c * 128 : (c + 1) * 128],
                                strip[:, ktl : ktl + 2, mb * 512 : (mb + 1) * 512],
                                start=(kt == 0),
                                stop=False,
                                perf_mode=mybir.MatmulPerfMode.DoubleRow,
                            )
                kt0 += cn

            # ---- bf16 phase: kt in [K1, KT)
            for ci, cn in enumerate(CHB):
                kb0 = kt0 - K1
                hsl = slice(kb0, kb0 + cn)
                nc.gpsimd.dma_start(hwb_big[:, hsl, :], HWB[:, hsl, :])
                strip = stripbp.tile([128, 8, RB], BF16, tag="sb")
                for s0 in range(0, cn, 4):
                    sn = min(4, cn - s0)
                    nc.gpsimd.dma_start(
                        strip[:, s0 : s0 + sn, :], Sb[:, kb0 + s0 : kb0 + s0 + sn, :]
                    )
                last = ci == len(CHB) - 1
                if not last:
                    for ktl in range(cn):
                        for c in range(4):
                            for mb in range(2):
                                nc.tensor.matmul(
                                    accs[mb][:, c * 512 : (c + 1) * 512],
                                    hwb_big[:, kb0 + ktl, c * 128 : (c + 1) * 128],
                                    strip[:, ktl, mb * 512 : (mb + 1) * 512],
                                    start=False,
                                    stop=False,
                                )
                else:
                    # Bank-major: banks stop staggered; tanh + store overlap
                    # the remaining banks' matmuls.
                    for c in range(4):
                        for mb in range(2):
                            for ktl in range(cn):
                                nc.tensor.matmul(
                                    accs[mb][:, c * 512 : (c + 1) * 512],
                                    hwb_big[:, kb0 + ktl, c * 128 : (c + 1) * 128],
                                    strip[:, ktl, mb * 512 : (mb + 1) * 512],
                                    start=False,
                                    stop=(ktl == cn - 1),
                                )
                            res = resp.tile([128, 512], BF16, tag="res")
                            nc.scalar.activation(
                                res[:],
                                accs[mb][:, c * 512 : (c + 1) * 512],
                                mybir.ActivationFunctionType.Tanh,
                                bias=b_t[:, c : c + 1],
                                scale=float(1.0 / (SC * SC)),
                            )
                            blk = (c * 2 + mb) * 512
                            nc.sync.dma_start(Out[:, blk : blk + 512], res[:])
                kt0 += cn

    nc.compile()
    return nc


def kernel(H, adj_matrix, W, b):
    global _CACHED_NC
    H = np.ascontiguousarray(np.asarray(H, dtype=np.float32))
    adj = np.ascontiguousarray(np.asarray(adj_matrix, dtype=np.float32))
    W = np.asarray(W, dtype=np.float32)
    b = np.asarray(b, dtype=np.float32)

    K1R = K1 * 128  # fp8 k-range in rows

    # Degrees (with self loop), scales
    deg = adj.sum(axis=0, dtype=np.float32) + 1.0
    d = deg.astype(np.float32) ** -0.5
    d = np.where(np.isinf(d), np.float32(0.0), d).astype(np.float32)

    # W folded into H (f32 BLAS), then column scale d and prescale SC
    HsW = (d[:, None] * (H @ W.T)).astype(np.float32)
    HsWs = HsW * np.float32(SC)
    HW8p = (
        HsWs[:K1R].reshape(K1, 128, NOUT).transpose(1, 0, 2)
    ).astype(NP8)
    HWBp = (
        HsWs[K1R:].reshape(KB, 128, NOUT).transpose(1, 0, 2)
    ).astype(NPBF)

    # Per-column mean of S'' over the fp8 k-range (exact, f32):
    # c_m = d_m * (sum_k<K1R adj[m,k] + [m<K1R]) / K1R
    rowsum = adj[:, :K1R].sum(axis=1, dtype=np.float32)
    rowsum[:K1R] += 1.0
    c = (d * rowsum / np.float32(K1R)).astype(np.float32)
    colsum = HsW[:K1R].sum(axis=0, dtype=np.float32)

    # Fold the mean-restore (c_m (x) colsum_n) into the bf16-phase S data:
    # find least-norm w with Hbf_device^T w = colsum (Hbf_device = the
    # quantized bf16 H the device actually contracts with), then add
    # c_m * w_k to the bf16 S block. The bf16 contraction then reproduces
    # the rank-1 correction exactly — no device-side correction matmuls.
    fSC = np.float32(SC)
    Hbf_dev = (HsWs[K1R:].astype(NPBF).astype(np.float32)) / fSC  # [KBR, NOUT]
    w = np.linalg.lstsq(Hbf_dev.T, colsum, rcond=None)[0].astype(np.float32)

    # S''^T via cache-blocked transpose: fp8 rows (centered, x SC) and bf16
    # rows (x SC, with the c (x) w fold), then exact self-loop diagonals.
    adjT8 = np.empty((K1R, N), dtype=NP8)
    adjTb = np.empty((N - K1R, N), dtype=NPBF)
    BLK = 256
    for i in range(0, N, BLK):
        blk = adj[i : i + BLK, :] * d[i : i + BLK, None]  # [m, k] = S''[k,m]^T
        adjT8[:, i : i + BLK] = ((blk[:, :K1R] - c[i : i + BLK, None]) * fSC).T.astype(
            NP8
        )
        adjTb[:, i : i + BLK] = (
            (blk[:, K1R:] + c[i : i + BLK, None] * w[None, :]) * fSC
        ).T.astype(NPBF)
    idx = np.arange(K1R)
    adjT8[idx, idx] = ((d[:K1R] * (adj[idx, idx] + 1.0) - c[:K1R]) * fSC).astype(NP8)
    idxb = np.arange(K1R, N)
    adjTb[idxb - K1R, idxb] = (
        (d[K1R:] * (adj[idxb, idxb] + 1.0) + c[K1R:] * w[idxb - K1R]) * fSC
    ).astype(NPBF)

    Bt = np.ascontiguousarray(b.reshape(4, 128).T)

    in_maps = []
    for cc in range(NC):
        r0, r1 = cc * RB, (cc + 1) * RB
        X8 = adjT8[:, r0:r1].reshape(K1, 128, RB).transpose(1, 0, 2)
        Xb = adjTb[:, r0:r1].reshape(KB, 128, RB).transpose(1, 0, 2)
        in_maps.append(
            {
                "S8": np.ascontiguousarray(X8),
                "Sb": np.ascontiguousarray(Xb),
                "HW8": HW8p,
                "HWB": HWBp,
                "Bt": Bt,
            }
        )

    if _CACHED_NC is None:
        _CACHED_NC = _build()
    globals()["_LAST_IN_MAPS"] = in_maps
    res = run_bass_kernel_spmd(_CACHED_NC, in_maps, core_ids=list(range(NC)))

    out = np.empty((N, NOUT), dtype=np.float32)
    for cc in range(NC):
        r0 = cc * RB
        X = res.results[cc]["out"].reshape(128, 4, 2, 512)
        out[r0 : r0 + RB, :] = (
            X.transpose(2, 3, 1, 0).reshape(RB, NOUT).astype(np.float32)
        )
    return out


# revision 42
# speedup vs baseline: 1.0141x; 1.0141x over previous
"""GCN layer on 8 trn2 NeuronCores.

out = tanh( (D^-1/2 (adj+I) D^-1/2) @ H @ W.T + b ), N=8192, nin=nout=512.

Associativity + normalization folding: with d = deg^-0.5,
  out = tanh( S''^T @ HsW + b )  where
  S''[k, m] = d_m * (adj + I)[m, k]   (fully-normalized adjacency, on host)
  HsW[k, :] = d_k * (H @ W.T)[k, :]   (W folded into H on host: one small
                                       4.3-GFLOP BLAS gemm)
so the device runs a SINGLE big gemm (8192x1024x512 per core) plus a fused
bias+tanh activation per PSUM bank. No second gemm, no transposes, no
PSUM->SBUF copies.

Mixed-precision contraction: the first K1=56 k-tiles run in fp8-e4m3 with
perf_mode=DoubleRow (2 k-tiles per matmul at the same 216ns issue cadence =
2x PE throughput); the last 8 k-tiles run in bf16. The fp8 operand S'' is
mean-centered per output column (c_m = mean_k S''[k,m]) before quantization
— halving its quantization error — and the mean contribution (rank-1 term
c_m (x) colsum(HsW)) is folded back into the bf16-phase S data on host: a
least-norm w with Hbf^T w = colsum is added as c (x) w, so the ordinary
bf16 contraction restores the mean with zero extra device work. Both
operands are pre-scaled by 512 (values ~0.0156 sit in e4m3's subnormal
range); the bf16 operands carry the same scales so all products share the
2^18 factor, removed by the final activation's scale. Measured end-to-end
rel err 1.753e-2 (L2, bit-stable across runs) vs the 2e-2 gate; max abs
err ~1.4e-3.

Sharding: output rows (and adj rows) split across 8 cores, 1024 rows each.
Output lands transposed ([nout, m] blocks); the host transposes it back.

All SWDGE DMAs drain FIFO on one logical queue, so issue order is arrival
order: HsW slices are interleaved with the S strip chunks in exactly the
order the k-loop consumes them, and the first chunks are small to cut
startup latency. Short dummy matmuls on scratch warm the PE's HAM clock
gate during the initial DMA wait. The last chunk runs bank-major so banks
stop staggered and the tanh+store tail overlaps the remaining matmuls.
"""

import sys

sys.path.insert(0, "/opt/trn_rl_repo")

import numpy as np
import ml_dtypes

from concourse import bass, bacc, tile, mybir
from concourse.bass_utils import run_bass_kernel_spmd

N = 8192
NIN = 512
NOUT = 512
NC = 8
RB = N // NC  # 1024 rows per core
KT = N // 128  # 64 k-tiles
K1 = 56  # k-tiles in fp8 DoubleRow; rest in bf16
KB = KT - K1
CH8 = [2, 2, 4] + [8] * 6  # fp8-phase chunk sizes (sum = K1, all even)
CHB = [8]  # bf16-phase chunk sizes (sum = KB)
SC = 512.0  # per-operand prescale (e4m3 subnormal floor is 2^-6)
F32 = mybir.dt.float32
BF16 = mybir.dt.bfloat16
FP8 = mybir.dt.float8e4
NPBF = ml_dtypes.bfloat16
NP8 = mybir.dt.np(mybir.dt.float8e4)

_CACHED_NC = None


def _build():
    nc = bacc.Bacc(None, target_bir_lowering=False)

    # Per-core inputs (packed layouts, see kernel() glue)
    S8 = nc.dram_tensor("S8", [128, K1, RB], FP8, kind="ExternalInput")
    Sb = nc.dram_tensor("Sb", [128, KB, RB], BF16, kind="ExternalInput")
    HW8 = nc.dram_tensor("HW8", [128, K1, NOUT], FP8, kind="ExternalInput")
    HWB = nc.dram_tensor("HWB", [128, KB, NOUT], BF16, kind="ExternalInput")
    Bt = nc.dram_tensor("Bt", [128, 4], F32, kind="ExternalInput")
    # Output transposed: col block (c*2+mb)*512 holds OutT[c-chunk, mb-half]
    Out = nc.dram_tensor("out", [128, 8 * 512], BF16, kind="ExternalOutput")

    with tile.TileContext(nc) as tc:
        with (
            tc.tile_pool(name="persist", bufs=1) as persist,
            tc.tile_pool(name="strip8", bufs=6) as strip8p,
            tc.tile_pool(name="stripb", bufs=2) as stripbp,
            tc.tile_pool(name="res", bufs=4) as resp,
            tc.tile_pool(name="acc", bufs=2, space=bass.MemorySpace.PSUM) as pacc,
        ):
            hw8_big = persist.tile([128, K1, NOUT], FP8)
            hwb_big = persist.tile([128, KB, NOUT], BF16)
            b_t = persist.tile([128, 4], F32)

            # Both m-halves accumulate across the whole k loop: 8 banks.
            acc0 = pacc.tile([128, 4 * 512], F32, tag="acc")
            acc1 = pacc.tile([128, 4 * 512], F32, tag="acc")
            accs = (acc0, acc1)

            # HAM warm-up (see module docstring).
            scratch = persist.tile([128, 128], BF16)
            nc.vector.memset(scratch[:], 0.0)
            for _ in range(44):
                nc.tensor.matmul(
                    acc0[:, 0:128], scratch[:], scratch[:], start=True, stop=True
                )

            # ---- fp8 DoubleRow phase: kt in [0, K1), 2 k-tiles per matmul
            kt0 = 0
            for ci, cn in enumerate(CH8):
                hsl = slice(kt0, kt0 + cn)
                nc.gpsimd.dma_start(hw8_big[:, hsl, :], HW8[:, hsl, :])
                strip = strip8p.tile([128, 8, RB], FP8, tag="s8")
                for s0 in range(0, cn, 4):
                    sn = min(4, cn - s0)
                    nc.gpsimd.dma_start(
                        strip[:, s0 : s0 + sn, :], S8[:, kt0 + s0 : kt0 + s0 + sn, :]
                    )
                if ci == 1:
                    nc.gpsimd.dma_start(b_t[:], Bt[:, :])
                for ktl in range(0, cn, 2):
                    kt = kt0 + ktl
                    for c in range(4):
                        for mb in range(2):
                            nc.tensor.matmul(
                                accs[mb][:, c * 512 : (c + 1) * 512],
                                hw8_big[:, ktl + kt0 : ktl + kt0 + 2, c * 128 : (c + 1) * 128],
                                strip[:, ktl : ktl + 2, mb * 512 : (mb + 1) * 512],
                                start=(kt == 0),
                                stop=False,
                                perf_mode=mybir.MatmulPerfMode.DoubleRow,
                            )
                kt0 += cn

            # ---- bf16 phase: kt in [K1, KT)
            for ci, cn in enumerate(CHB):
                kb0 = kt0 - K1
                hsl = slice(kb0, kb0 + cn)
                nc.gpsimd.dma_start(hwb_big[:, hsl, :], HWB[:, hsl, :])
                strip = stripbp.tile([128, 8, RB], BF16, tag="sb")
                for s0 in range(0, cn, 4):
                    sn = min(4, cn - s0)
                    nc.gpsimd.dma_start(
                        strip[:, s0 : s0 + sn, :], Sb[:, kb0 + s0 : kb0 + s0 + sn, :]
                    )
                last = ci == len(CHB) - 1
                if not last:
                    for ktl in range(cn):
                        for c in range(4):
                            for mb in range(2):
                                nc.tensor.matmul(
                                    accs[mb][:, c * 512 : (c + 1) * 512],
                                    hwb_big[:, kb0 + ktl, c * 128 : (c + 1) * 128],
                                    strip[:, ktl, mb * 512 : (mb + 1) * 512],
                                    start=False,
                                    stop=False,
                                )
                else:
                    # Bank-major: banks stop staggered; tanh + store overlap
                    # the remaining banks' matmuls.
                    for c in range(4):
                        for mb in range(2):
                            for ktl in range(cn):
                                nc.tensor.matmul(
                                    accs[mb][:, c * 512 : (c + 1) * 512],
                                    hwb_big[:, kb0 + ktl, c * 128 : (c + 1) * 128],
                                    strip[:, ktl, mb * 512 : (mb + 1) * 512],
                                    start=False,
                                    stop=(ktl == cn - 1),
                                )
                            res = resp.tile([128, 512], BF16, tag="res")
                            nc.scalar.activation(
                                res[:],
                                accs[mb][:, c * 512 : (c + 1) * 512],
                                mybir.ActivationFunctionType.Tanh,
                                bias=b_t[:, c : c + 1],
                                scale=float(1.0 / (SC * SC)),
                            )
                            blk = (c * 2 + mb) * 512
                            nc.sync.dma_start(Out[:, blk : blk + 512], res[:])
                kt0 += cn

    nc.compile()
    return nc


def kernel(H, adj_matrix, W, b):
    global _CACHED_NC
    H = np.ascontiguousarray(np.asarray(H, dtype=np.float32))
    adj = np.ascontiguousarray(np.asarray(adj_matrix, dtype=np.float32))
    W = np.asarray(W, dtype=np.float32)
    b = np.asarray(b, dtype=np.float32)

    K1R = K1 * 128  # fp8 k-range in rows

    # Degrees (with self loop), scales
    deg = adj.sum(axis=0, dtype=np.float32) + 1.0
    d = deg.astype(np.float32) ** -0.5
    d = np.where(np.isinf(d), np.float32(0.0), d).astype(np.float32)

    # W folded into H (f32 BLAS), then column scale d and prescale SC
    HsW = (d[:, None] * (H @ W.T)).astype(np.float32)
    HsWs = HsW * np.float32(SC)
    HW8p = (
        HsWs[:K1R].reshape(K1, 128, NOUT).transpose(1, 0, 2)
    ).astype(NP8)
    HWBp = (
        HsWs[K1R:].reshape(KB, 128, NOUT).transpose(1, 0, 2)
    ).astype(NPBF)

    # Per-column mean of S'' over the fp8 k-range (exact, f32):
    # c_m = d_m * (sum_k<K1R adj[m,k] + [m<K1R]) / K1R
    rowsum = adj[:, :K1R].sum(axis=1, dtype=np.float32)
    rowsum[:K1R] += 1.0
    c = (d * rowsum / np.float32(K1R)).astype(np.float32)
    colsum = HsW[:K1R].sum(axis=0, dtype=np.float32)

    # Fold the mean-restore (c_m (x) colsum_n) into the bf16-phase S data:
    # find least-norm w with Hbf_device^T w = colsum (Hbf_device = the
    # quantized bf16 H the device actually contracts with), then add
    # c_m * w_k to the bf16 S block. The bf16 contraction then reproduces
    # the rank-1 correction exactly — no device-side correction matmuls.
    fSC = np.float32(SC)
    Hbf_dev = (HsWs[K1R:].astype(NPBF).astype(np.float32)) / fSC  # [KBR, NOUT]
    w = np.linalg.lstsq(Hbf_dev.T, colsum, rcond=None)[0].astype(np.float32)

    # S''^T via cache-blocked transpose: fp8 rows (centered, x SC) and bf16
    # rows (x SC, with the c (x) w fold), then exact self-loop diagonals.
    adjT8 = np.empty((K1R, N), dtype=NP8)
    adjTb = np.empty((N - K1R, N), dtype=NPBF)
    BLK = 256
    for i in range(0, N, BLK):
        blk = adj[i : i + BLK, :] * d[i : i + BLK, None]  # [m, k] = S''[k,m]^T
        adjT8[:, i : i + BLK] = ((blk[:, :K1R] - c[i : i + BLK, None]) * fSC).T.astype(
            NP8
        )
        adjTb[:, i : i + BLK] = (
            (blk[:, K1R:] + c[i : i + BLK, None] * w[None, :]) * fSC
        ).T.astype(NPBF)
    idx = np.arange(K1R)
    adjT8[idx, idx] = ((d[:K1R] * (adj[idx, idx] + 1.0) - c[:K1R]) * fSC).astype(NP8)
    idxb = np.arange(K1R, N)
    adjTb[idxb - K1R, idxb] = (
        (d[K1R:] * (adj[idxb, idxb] + 1.0) + c[K1R:] * w[idxb - K1R]) * fSC
    ).astype(NPBF)

    Bt = np.ascontiguousarray(b.reshape(4, 128).T)

    in_maps = []
    for cc in range(NC):
        r0, r1 = cc * RB, (cc + 1) * RB
        X8 = adjT8[:, r0:r1].reshape(K1, 128, RB).transpose(1, 0, 2)
        Xb = adjTb[:, r0:r1].reshape(KB, 128, RB).transpose(1, 0, 2)
        in_maps.append(
            {
                "S8": np.ascontiguousarray(X8),
                "Sb": np.ascontiguousarray(Xb),
                "HW8": HW8p,
                "HWB": HWBp,
                "Bt": Bt,
            }
        )

    if _CACHED_NC is None:
        _CACHED_NC = _build()
    globals()["_LAST_IN_MAPS"] = in_maps
    res = run_bass_kernel_spmd(_CACHED_NC, in_maps, core_ids=list(range(NC)))

    out = np.empty((N, NOUT), dtype=np.float32)
    for cc in range(NC):
        r0 = cc * RB
        X = res.results[cc]["out"].reshape(128, 4, 2, 512)
        out[r0 : r0 + RB, :] = (
            X.transpose(2, 3, 1, 0).reshape(RB, NOUT).astype(np.float32)
        )
    return out


# revision 44
# speedup vs baseline: 1.0322x; 1.0178x over previous
"""GCN layer on 8 trn2 NeuronCores.

out = tanh( (D^-1/2 (adj+I) D^-1/2) @ H @ W.T + b ), N=8192, nin=nout=512.

Associativity + normalization folding: with d = deg^-0.5,
  out = tanh( S''^T @ HsW + b )  where
  S''[k, m] = d_m * (adj + I)[m, k]   (fully-normalized adjacency, on host)
  HsW[k, :] = d_k * (H @ W.T)[k, :]   (W folded into H on host: one small
                                       4.3-GFLOP BLAS gemm)
so the device runs a SINGLE big gemm (8192x1024x512 per core) plus a fused
bias+tanh activation per PSUM bank. No second gemm, no transposes, no
PSUM->SBUF copies.

Mixed-precision contraction: the first K1=56 k-tiles run in fp8-e4m3 with
perf_mode=DoubleRow (2 k-tiles per matmul at the same 216ns issue cadence =
2x PE throughput); the last 8 k-tiles run in bf16. The fp8 operand S'' is
mean-centered per output column (c_m = mean_k S''[k,m]) before quantization
— halving its quantization error — and the mean contribution (rank-1 term
c_m (x) colsum(HsW)) is folded back into the bf16-phase S data on host: a
least-norm w with Hbf^T w = colsum is added as c (x) w, so the ordinary
bf16 contraction restores the mean with zero extra device work. Both
operands are pre-scaled by 512 (values ~0.0156 sit in e4m3's subnormal
range); the bf16 operands carry the same scales so all products share the
2^18 factor, removed by the final activation's scale. Measured end-to-end
rel err 1.753e-2 (L2, bit-stable across runs) vs the 2e-2 gate; max abs
err ~1.4e-3.

Sharding: output rows (and adj rows) split across 8 cores, 1024 rows each.
Output lands transposed ([nout, m] blocks); the host transposes it back.

All SWDGE DMAs drain FIFO on one logical queue, so issue order is arrival
order: HsW slices are interleaved with the S strip chunks in exactly the
order the k-loop consumes them, and the first chunks are small to cut
startup latency. Short dummy matmuls on scratch warm the PE's HAM clock
gate during the initial DMA wait. The last chunk runs bank-major so banks
stop staggered and the tanh+store tail overlaps the remaining matmuls.
"""

import sys

sys.path.insert(0, "/opt/trn_rl_repo")

import numpy as np
import ml_dtypes

from concourse import bass, bacc, tile, mybir
from concourse.bass_utils import run_bass_kernel_spmd

N = 8192
NIN = 512
NOUT = 512
NC = 8
RB = N // NC  # 1024 rows per core
KT = N // 128  # 64 k-tiles
K1 = 56  # k-tiles in fp8 DoubleRow; rest in bf16
KB = KT - K1
CH8 = [2, 2, 4] + [8] * 6  # fp8-phase chunk sizes (sum = K1, all even)
CHB = [8]  # bf16-phase chunk sizes (sum = KB)
SC = 512.0  # per-operand prescale (e4m3 subnormal floor is 2^-6)
F32 = mybir.dt.float32
BF16 = mybir.dt.bfloat16
FP8 = mybir.dt.float8e4
NPBF = ml_dtypes.bfloat16
NP8 = mybir.dt.np(mybir.dt.float8e4)

_CACHED_NC = None


def _build():
    nc = bacc.Bacc(None, target_bir_lowering=False)

    # Per-core inputs (packed layouts, see kernel() glue)
    S8 = nc.dram_tensor("S8", [128, K1, RB], FP8, kind="ExternalInput")
    Sb = nc.dram_tensor("Sb", [128, KB, RB], BF16, kind="ExternalInput")
    HW8 = nc.dram_tensor("HW8", [128, K1, NOUT], FP8, kind="ExternalInput")
    HWB = nc.dram_tensor("HWB", [128, KB, NOUT], BF16, kind="ExternalInput")
    Bt = nc.dram_tensor("Bt", [128, 4], F32, kind="ExternalInput")
    # Output transposed: col block (c*2+mb)*512 holds OutT[c-chunk, mb-half]
    Out = nc.dram_tensor("out", [128, 8 * 512], BF16, kind="ExternalOutput")

    with tile.TileContext(nc) as tc:
        with (
            tc.tile_pool(name="persist", bufs=1) as persist,
            tc.tile_pool(name="strip8", bufs=6) as strip8p,
            tc.tile_pool(name="stripb", bufs=2) as stripbp,
            tc.tile_pool(name="res", bufs=4) as resp,
            tc.tile_pool(name="acc", bufs=2, space=bass.MemorySpace.PSUM) as pacc,
        ):
            hw8_big = persist.tile([128, K1, NOUT], FP8)
            hwb_big = persist.tile([128, KB, NOUT], BF16)
            b_t = persist.tile([128, 4], F32)

            # Both m-halves accumulate across the whole k loop: 8 banks.
            acc0 = pacc.tile([128, 4 * 512], F32, tag="acc")
            acc1 = pacc.tile([128, 4 * 512], F32, tag="acc")
            accs = (acc0, acc1)

            # HAM warm-up (see module docstring). N=64 keeps each dummy near
            # the NX dispatch floor (~30-55ns) so the warm-up span is
            # insensitive to the HAM clock phase; the bank is overwritten by
            # the real start=True matmul.
            scratch = persist.tile([128, 128], BF16)
            nc.vector.memset(scratch[:], 0.0)
            for _ in range(100):
                nc.tensor.matmul(
                    acc0[:, 0:64], scratch[:], scratch[:, 0:64], start=True, stop=True
                )

            # ---- fp8 DoubleRow phase: kt in [0, K1), 2 k-tiles per matmul
            kt0 = 0
            for ci, cn in enumerate(CH8):
                hsl = slice(kt0, kt0 + cn)
                nc.gpsimd.dma_start(hw8_big[:, hsl, :], HW8[:, hsl, :])
                strip = strip8p.tile([128, 8, RB], FP8, tag="s8")
                for s0 in range(0, cn, 4):
                    sn = min(4, cn - s0)
                    nc.gpsimd.dma_start(
                        strip[:, s0 : s0 + sn, :], S8[:, kt0 + s0 : kt0 + s0 + sn, :]
                    )
                if ci == 1:
                    nc.gpsimd.dma_start(b_t[:], Bt[:, :])
                for ktl in range(0, cn, 2):
                    kt = kt0 + ktl
                    for c in range(4):
                        for mb in range(2):
                            nc.tensor.matmul(
                                accs[mb][:, c * 512 : (c + 1) * 512],
                                hw8_big[:, ktl + kt0 : ktl + kt0 + 2, c * 128 : (c + 1) * 128],
                                strip[:, ktl : ktl + 2, mb * 512 : (mb + 1) * 512],
                                start=(kt == 0),
                                stop=False,
                                perf_mode=mybir.MatmulPerfMode.DoubleRow,
                            )
                kt0 += cn

            # ---- bf16 phase: kt in [K1, KT)
            for ci, cn in enumerate(CHB):
                kb0 = kt0 - K1
                hsl = slice(kb0, kb0 + cn)
                nc.gpsimd.dma_start(hwb_big[:, hsl, :], HWB[:, hsl, :])
                strip = stripbp.tile([128, 8, RB], BF16, tag="sb")
                for s0 in range(0, cn, 4):
                    sn = min(4, cn - s0)
                    nc.gpsimd.dma_start(
                        strip[:, s0 : s0 + sn, :], Sb[:, kb0 + s0 : kb0 + s0 + sn, :]
                    )
                last = ci == len(CHB) - 1
                if not last:
                    for ktl in range(cn):
                        for c in range(4):
                            for mb in range(2):
                                nc.tensor.matmul(
                                    accs[mb][:, c * 512 : (c + 1) * 512],
                                    hwb_big[:, kb0 + ktl, c * 128 : (c + 1) * 128],
                                    strip[:, ktl, mb * 512 : (mb + 1) * 512],
                                    start=False,
                                    stop=False,
                                )
                else:
                    # Bank-major: banks stop staggered; tanh + store overlap
                    # the remaining banks' matmuls.
                    for c in range(4):
                        for mb in range(2):
                            for ktl in range(cn):
                                nc.tensor.matmul(
                                    accs[mb][:, c * 512 : (c + 1) * 512],
                                    hwb_big[:, kb0 + ktl, c * 128 : (c + 1) * 128],
                                    strip[:, ktl, mb * 512 : (mb + 1) * 512],
                                    start=False,
                                    stop=(ktl == cn - 1),
                                )
                            res = resp.tile([128, 512], BF16, tag="res")
                            nc.scalar.activation(
                                res[:],
                                accs[mb][:, c * 512 : (c + 1) * 512],
                                mybir.ActivationFunctionType.Tanh,
                                bias=b_t[:, c : c + 1],
                                scale=float(1.0 / (SC * SC)),
                            )
                            blk = (c * 2 + mb) * 512
                            nc.sync.dma_start(Out[:, blk : blk + 512], res[:])
                kt0 += cn

    nc.compile()
    return nc


def kernel(H, adj_matrix, W, b):
    global _CACHED_NC
    H = np.ascontiguousarray(np.asarray(H, dtype=np.float32))
    adj = np.ascontiguousarray(np.asarray(adj_matrix, dtype=np.float32))
    W = np.asarray(W, dtype=np.float32)
    b = np.asarray(b, dtype=np.float32)

    K1R = K1 * 128  # fp8 k-range in rows

    # Degrees (with self loop), scales
    deg = adj.sum(axis=0, dtype=np.float32) + 1.0
    d = deg.astype(np.float32) ** -0.5
    d = np.where(np.isinf(d), np.float32(0.0), d).astype(np.float32)

    # W folded into H (f32 BLAS), then column scale d and prescale SC
    HsW = (d[:, None] * (H @ W.T)).astype(np.float32)
    HsWs = HsW * np.float32(SC)
    HW8p = (
        HsWs[:K1R].reshape(K1, 128, NOUT).transpose(1, 0, 2)
    ).astype(NP8)
    HWBp = (
        HsWs[K1R:].reshape(KB, 128, NOUT).transpose(1, 0, 2)
    ).astype(NPBF)

    # Per-column mean of S'' over the fp8 k-range (exact, f32):
    # c_m = d_m * (sum_k<K1R adj[m,k] + [m<K1R]) / K1R
    rowsum = adj[:, :K1R].sum(axis=1, dtype=np.float32)
    rowsum[:K1R] += 1.0
    c = (d * rowsum / np.float32(K1R)).astype(np.float32)
    colsum = HsW[:K1R].sum(axis=0, dtype=np.float32)

    # Fold the mean-restore (c_m (x) colsum_n) into the bf16-phase S data:
    # find least-norm w with Hbf_device^T w = colsum (Hbf_device = the
    # quantized bf16 H the device actually contracts with), then add
    # c_m * w_k to the bf16 S block. The bf16 contraction then reproduces
    # the rank-1 correction exactly — no device-side correction matmuls.
    fSC = np.float32(SC)
    Hbf_dev = (HsWs[K1R:].astype(NPBF).astype(np.float32)) / fSC  # [KBR, NOUT]
    w = np.linalg.lstsq(Hbf_dev.T, colsum, rcond=None)[0].astype(np.float32)

    # S''^T via cache-blocked transpose: fp8 rows (centered, x SC) and bf16
    # rows (x SC, with the c (x) w fold), then exact self-loop diagonals.
    adjT8 = np.empty((K1R, N), dtype=NP8)
    adjTb = np.empty((N - K1R, N), dtype=NPBF)
    BLK = 256
    for i in range(0, N, BLK):
        blk = adj[i : i + BLK, :] * d[i : i + BLK, None]  # [m, k] = S''[k,m]^T
        adjT8[:, i : i + BLK] = ((blk[:, :K1R] - c[i : i + BLK, None]) * fSC).T.astype(
            NP8
        )
        adjTb[:, i : i + BLK] = (
            (blk[:, K1R:] + c[i : i + BLK, None] * w[None, :]) * fSC
        ).T.astype(NPBF)
    idx = np.arange(K1R)
    adjT8[idx, idx] = ((d[:K1R] * (adj[idx, idx] + 1.0) - c[:K1R]) * fSC).astype(NP8)
    idxb = np.arange(K1R, N)
    adjTb[idxb - K1R, idxb] = (
        (d[K1R:] * (adj[idxb, idxb] + 1.0) + c[K1R:] * w[idxb - K1R]) * fSC
    ).astype(NPBF)

    Bt = np.ascontiguousarray(b.reshape(4, 128).T)

    in_maps = []
    for cc in range(NC):
        r0, r1 = cc * RB, (cc + 1) * RB
        X8 = adjT8[:, r0:r1].reshape(K1, 128, RB).transpose(1, 0, 2)
        Xb = adjTb[:, r0:r1].reshape(KB, 128, RB).transpose(1, 0, 2)
        in_maps.append(
            {
                "S8": np.ascontiguousarray(X8),
                "Sb": np.ascontiguousarray(Xb),
                "HW8": HW8p,
                "HWB": HWBp,
                "Bt": Bt,
            }
        )

    if _CACHED_NC is None:
        _CACHED_NC = _build()
    globals()["_LAST_IN_MAPS"] = in_maps
    res = run_bass_kernel_spmd(_CACHED_NC, in_maps, core_ids=list(range(NC)))

    out = np.empty((N, NOUT), dtype=np.float32)
    for cc in range(NC):
        r0 = cc * RB
        X = res.results[cc]["out"].reshape(128, 4, 2, 512)
        out[r0 : r0 + RB, :] = (
            X.transpose(2, 3, 1, 0).reshape(RB, NOUT).astype(np.float32)
        )
    return out


# revision 45
# speedup vs baseline: 1.0420x; 1.0095x over previous
"""GCN layer on 8 trn2 NeuronCores.

out = tanh( (D^-1/2 (adj+I) D^-1/2) @ H @ W.T + b ), N=8192, nin=nout=512.

Associativity + normalization folding: with d = deg^-0.5,
  out = tanh( S''^T @ HsW + b )  where
  S''[k, m] = d_m * (adj + I)[m, k]   (fully-normalized adjacency, on host)
  HsW[k, :] = d_k * (H @ W.T)[k, :]   (W folded into H on host: one small
                                       4.3-GFLOP BLAS gemm)
so the device runs a SINGLE big gemm (8192x1024x512 per core) plus a fused
bias+tanh activation per PSUM bank. No second gemm, no transposes, no
PSUM->SBUF copies.

Mixed-precision contraction: the first K1=56 k-tiles run in fp8-e4m3 with
perf_mode=DoubleRow (2 k-tiles per matmul at the same 216ns issue cadence =
2x PE throughput); the last 8 k-tiles run in bf16. The fp8 operand S'' is
mean-centered per output column (c_m = mean_k S''[k,m]) before quantization
— halving its quantization error — and the mean contribution (rank-1 term
c_m (x) colsum(HsW)) is folded back into the bf16-phase S data on host: a
least-norm w with Hbf^T w = colsum is added as c (x) w, so the ordinary
bf16 contraction restores the mean with zero extra device work. Both
operands are pre-scaled by 512 (values ~0.0156 sit in e4m3's subnormal
range); the bf16 operands carry the same scales so all products share the
2^18 factor, removed by the final activation's scale. Measured end-to-end
rel err 1.753e-2 (L2, bit-stable across runs) vs the 2e-2 gate; max abs
err ~1.4e-3.

Sharding: output rows (and adj rows) split across 8 cores, 1024 rows each.
Output lands transposed ([nout, m] blocks); the host transposes it back.

All SWDGE DMAs drain FIFO on one logical queue, so issue order is arrival
order: HsW slices are interleaved with the S strip chunks in exactly the
order the k-loop consumes them, and the first chunks are small to cut
startup latency. Short dummy matmuls on scratch warm the PE's HAM clock
gate during the initial DMA wait. The last chunk runs bank-major so banks
stop staggered and the tanh+store tail overlaps the remaining matmuls.
"""

import sys

sys.path.insert(0, "/opt/trn_rl_repo")

import numpy as np
import ml_dtypes

from concourse import bass, bacc, tile, mybir
from concourse.bass_utils import run_bass_kernel_spmd

N = 8192
NIN = 512
NOUT = 512
NC = 8
RB = N // NC  # 1024 rows per core
KT = N // 128  # 64 k-tiles
K1 = 58  # k-tiles in fp8 DoubleRow; rest in bf16
KB = KT - K1
CH8 = [2, 2, 4] + [8] * 6 + [2]  # fp8-phase chunk sizes (sum = K1, all even)
CHB = [6]  # bf16-phase chunk sizes (sum = KB)
SC = 512.0  # per-operand prescale (e4m3 subnormal floor is 2^-6)
F32 = mybir.dt.float32
BF16 = mybir.dt.bfloat16
FP8 = mybir.dt.float8e4
NPBF = ml_dtypes.bfloat16
NP8 = mybir.dt.np(mybir.dt.float8e4)

_CACHED_NC = None


def _build():
    nc = bacc.Bacc(None, target_bir_lowering=False)

    # Per-core inputs (packed layouts, see kernel() glue)
    S8 = nc.dram_tensor("S8", [128, K1, RB], FP8, kind="ExternalInput")
    Sb = nc.dram_tensor("Sb", [128, KB, RB], BF16, kind="ExternalInput")
    HW8 = nc.dram_tensor("HW8", [128, K1, NOUT], FP8, kind="ExternalInput")
    HWB = nc.dram_tensor("HWB", [128, KB, NOUT], BF16, kind="ExternalInput")
    Bt = nc.dram_tensor("Bt", [128, 4], F32, kind="ExternalInput")
    # Output transposed: col block (c*2+mb)*512 holds OutT[c-chunk, mb-half]
    Out = nc.dram_tensor("out", [128, 8 * 512], BF16, kind="ExternalOutput")

    with tile.TileContext(nc) as tc:
        with (
            tc.tile_pool(name="persist", bufs=1) as persist,
            tc.tile_pool(name="strip8", bufs=6) as strip8p,
            tc.tile_pool(name="stripb", bufs=2) as stripbp,
            tc.tile_pool(name="res", bufs=4) as resp,
            tc.tile_pool(name="acc", bufs=2, space=bass.MemorySpace.PSUM) as pacc,
        ):
            hw8_big = persist.tile([128, K1, NOUT], FP8)
            hwb_big = persist.tile([128, KB, NOUT], BF16)
            b_t = persist.tile([128, 4], F32)

            # Both m-halves accumulate across the whole k loop: 8 banks.
            acc0 = pacc.tile([128, 4 * 512], F32, tag="acc")
            acc1 = pacc.tile([128, 4 * 512], F32, tag="acc")
            accs = (acc0, acc1)

            # HAM warm-up (see module docstring). N=64 keeps each dummy near
            # the NX dispatch floor (~30-55ns) so the warm-up span is
            # insensitive to the HAM clock phase; the bank is overwritten by
            # the real start=True matmul.
            scratch = persist.tile([128, 128], BF16)
            nc.vector.memset(scratch[:], 0.0)
            for _ in range(100):
                nc.tensor.matmul(
                    acc0[:, 0:64], scratch[:], scratch[:, 0:64], start=True, stop=True
                )

            # ---- fp8 DoubleRow phase: kt in [0, K1), 2 k-tiles per matmul
            kt0 = 0
            for ci, cn in enumerate(CH8):
                hsl = slice(kt0, kt0 + cn)
                nc.gpsimd.dma_start(hw8_big[:, hsl, :], HW8[:, hsl, :])
                strip = strip8p.tile([128, 8, RB], FP8, tag="s8")
                for s0 in range(0, cn, 4):
                    sn = min(4, cn - s0)
                    nc.gpsimd.dma_start(
                        strip[:, s0 : s0 + sn, :], S8[:, kt0 + s0 : kt0 + s0 + sn, :]
                    )
                if ci == 1:
                    nc.gpsimd.dma_start(b_t[:], Bt[:, :])
                for ktl in range(0, cn, 2):
                    kt = kt0 + ktl
                    for c in range(4):
                        for mb in range(2):
                            nc.tensor.matmul(
                                accs[mb][:, c * 512 : (c + 1) * 512],
                                hw8_big[:, ktl + kt0 : ktl + kt0 + 2, c * 128 : (c + 1) * 128],
                                strip[:, ktl : ktl + 2, mb * 512 : (mb + 1) * 512],
                                start=(kt == 0),
                                stop=False,
                                perf_mode=mybir.MatmulPerfMode.DoubleRow,
                            )
                kt0 += cn

            # ---- bf16 phase: kt in [K1, KT)
            for ci, cn in enumerate(CHB):
                kb0 = kt0 - K1
                hsl = slice(kb0, kb0 + cn)
                nc.gpsimd.dma_start(hwb_big[:, hsl, :], HWB[:, hsl, :])
                strip = stripbp.tile([128, 8, RB], BF16, tag="sb")
                for s0 in range(0, cn, 4):
                    sn = min(4, cn - s0)
                    nc.gpsimd.dma_start(
                        strip[:, s0 : s0 + sn, :], Sb[:, kb0 + s0 : kb0 + s0 + sn, :]
                    )
                last = ci == len(CHB) - 1
                if not last:
                    for ktl in range(cn):
                        for c in range(4):
                            for mb in range(2):
                                nc.tensor.matmul(
                                    accs[mb][:, c * 512 : (c + 1) * 512],
                                    hwb_big[:, kb0 + ktl, c * 128 : (c + 1) * 128],
                                    strip[:, ktl, mb * 512 : (mb + 1) * 512],
                                    start=False,
                                    stop=False,
                                )
                else:
                    # Bank-major: banks stop staggered; tanh + store overlap
                    # the remaining banks' matmuls.
                    for c in range(4):
                        for mb in range(2):
                            for ktl in range(cn):
                                nc.tensor.matmul(
                                    accs[mb][:, c * 512 : (c + 1) * 512],
                                    hwb_big[:, kb0 + ktl, c * 128 : (c + 1) * 128],
                                    strip[:, ktl, mb * 512 : (mb + 1) * 512],
                                    start=False,
                                    stop=(ktl == cn - 1),
                                )
                            res = resp.tile([128, 512], BF16, tag="res")
                            nc.scalar.activation(
                                res[:],
                                accs[mb][:, c * 512 : (c + 1) * 512],
                                mybir.ActivationFunctionType.Tanh,
                                bias=b_t[:, c : c + 1],
                                scale=float(1.0 / (SC * SC)),
                            )
                            blk = (c * 2 + mb) * 512
                            nc.sync.dma_start(Out[:, blk : blk + 512], res[:])
                kt0 += cn

    nc.compile()
    return nc


def kernel(H, adj_matrix, W, b):
    global _CACHED_NC
    H = np.ascontiguousarray(np.asarray(H, dtype=np.float32))
    adj = np.ascontiguousarray(np.asarray(adj_matrix, dtype=np.float32))
    W = np.asarray(W, dtype=np.float32)
    b = np.asarray(b, dtype=np.float32)

    K1R = K1 * 128  # fp8 k-range in rows

    # Degrees (with self loop), scales
    deg = adj.sum(axis=0, dtype=np.float32) + 1.0
    d = deg.astype(np.float32) ** -0.5
    d = np.where(np.isinf(d), np.float32(0.0), d).astype(np.float32)

    # W folded into H (f32 BLAS), then column scale d and prescale SC
    HsW = (d[:, None] * (H @ W.T)).astype(np.float32)
    HsWs = HsW * np.float32(SC)
    HW8p = (
        HsWs[:K1R].reshape(K1, 128, NOUT).transpose(1, 0, 2)
    ).astype(NP8)
    HWBp = (
        HsWs[K1R:].reshape(KB, 128, NOUT).transpose(1, 0, 2)
    ).astype(NPBF)

    # Per-column mean of S'' over the fp8 k-range (exact, f32):
    # c_m = d_m * (sum_k<K1R adj[m,k] + [m<K1R]) / K1R
    rowsum = adj[:, :K1R].sum(axis=1, dtype=np.float32)
    rowsum[:K1R] += 1.0
    c = (d * rowsum / np.float32(K1R)).astype(np.float32)
    colsum = HsW[:K1R].sum(axis=0, dtype=np.float32)

    # Fold the mean-restore (c_m (x) colsum_n) into the bf16-phase S data:
    # find least-norm w with Hbf_device^T w = colsum (Hbf_device = the
    # quantized bf16 H the device actually contracts with), then add
    # c_m * w_k to the bf16 S block. The bf16 contraction then reproduces
    # the rank-1 correction exactly — no device-side correction matmuls.
    fSC = np.float32(SC)
    Hbf_dev = (HsWs[K1R:].astype(NPBF).astype(np.float32)) / fSC  # [KBR, NOUT]
    w = np.linalg.lstsq(Hbf_dev.T, colsum, rcond=None)[0].astype(np.float32)

    # S''^T via cache-blocked transpose: fp8 rows (centered, x SC) and bf16
    # rows (x SC, with the c (x) w fold), then exact self-loop diagonals.
    adjT8 = np.empty((K1R, N), dtype=NP8)
    adjTb = np.empty((N - K1R, N), dtype=NPBF)
    BLK = 256
    for i in range(0, N, BLK):
        blk = adj[i : i + BLK, :] * d[i : i + BLK, None]  # [m, k] = S''[k,m]^T
        adjT8[:, i : i + BLK] = ((blk[:, :K1R] - c[i : i + BLK, None]) * fSC).T.astype(
            NP8
        )
        adjTb[:, i : i + BLK] = (
            (blk[:, K1R:] + c[i : i + BLK, None] * w[None, :]) * fSC
        ).T.astype(NPBF)
    idx = np.arange(K1R)
    adjT8[idx, idx] = ((d[:K1R] * (adj[idx, idx] + 1.0) - c[:K1R]) * fSC).astype(NP8)
    idxb = np.arange(K1R, N)
    adjTb[idxb - K1R, idxb] = (
        (d[K1R:] * (adj[idxb, idxb] + 1.0) + c[K1R:] * w[idxb - K1R]) * fSC
    ).astype(NPBF)

    Bt = np.ascontiguousarray(b.reshape(4, 128).T)

    in_maps = []
    for cc in range(NC):
        r0, r1 = cc * RB, (cc + 1) * RB
        X8 = adjT8[:, r0:r1].reshape(K1, 128, RB).transpose(1, 0, 2)
        Xb = adjTb[:, r0:r1].reshape(KB, 128, RB).transpose(1, 0, 2)
        in_maps.append(
            {
                "S8": np.ascontiguousarray(X8),
                "Sb": np.ascontiguousarray(Xb),
                "HW8": HW8p,
                "HWB": HWBp,
                "Bt": Bt,
            }
        )

    if _CACHED_NC is None:
        _CACHED_NC = _build()
    globals()["_LAST_IN_MAPS"] = in_maps
    res = run_bass_kernel_spmd(_CACHED_NC, in_maps, core_ids=list(range(NC)))

    out = np.empty((N, NOUT), dtype=np.float32)
    for cc in range(NC):
        r0 = cc * RB
        X = res.results[cc]["out"].reshape(128, 4, 2, 512)
        out[r0 : r0 + RB, :] = (
            X.transpose(2, 3, 1, 0).reshape(RB, NOUT).astype(np.float32)
        )
    return out


# revision 48
# speedup vs baseline: 1.0546x; 1.0121x over previous
"""GCN layer on 8 trn2 NeuronCores.

out = tanh( (D^-1/2 (adj+I) D^-1/2) @ H @ W.T + b ), N=8192, nin=nout=512.

Associativity + normalization folding: with d = deg^-0.5,
  out = tanh( S''^T @ HsW + b )  where
  S''[k, m] = d_m * (adj + I)[m, k]   (fully-normalized adjacency, on host)
  HsW[k, :] = d_k * (H @ W.T)[k, :]   (W folded into H on host: one small
                                       4.3-GFLOP BLAS gemm)
so the device runs a SINGLE big gemm (8192x1024x512 per core) plus a fused
bias+tanh activation per PSUM bank. No second gemm, no transposes, no
PSUM->SBUF copies.

Mixed-precision contraction: the first K1=58 k-tiles run in fp8-e4m3 with
perf_mode=DoubleRow (2 k-tiles per matmul at the same 216ns issue cadence =
2x PE throughput); the last 6 k-tiles run in bf16. The fp8 operand S'' is
mean-centered per output column (c_m = mean_k S''[k,m]) before quantization
— halving its quantization error — and the mean contribution (rank-1 term
c_m (x) colsum(HsW)) is folded back into the bf16-phase S data on host: a
least-norm w with Hbf^T w = colsum is added as c (x) w, so the ordinary
bf16 contraction restores the mean with zero extra device work. Both
operands are pre-scaled by 512 (values ~0.0156 sit in e4m3's subnormal
range); the bf16 operands carry the same scales so all products share the
2^18 factor, removed by the final activation's scale. Measured end-to-end
rel err 1.789e-2 (L2, bit-stable across runs) vs the 2e-2 gate; max abs
err ~1.4e-3.

Sharding: output rows (and adj rows) split across 8 cores, 1024 rows each.
Output lands transposed ([nout, m] blocks); the host transposes it back.

All SWDGE DMAs drain FIFO on one logical queue, so issue order is arrival
order: HsW slices are interleaved with the S strip chunks in exactly the
order the k-loop consumes them, and the first chunks are small to cut
startup latency. Short dummy matmuls on scratch warm the PE's HAM clock
gate during the initial DMA wait. The last chunk runs bank-major so banks
stop staggered and the tanh+store tail overlaps the remaining matmuls.
"""

import sys

sys.path.insert(0, "/opt/trn_rl_repo")

import numpy as np
import ml_dtypes

from concourse import bass, bacc, tile, mybir
from concourse.bass_utils import run_bass_kernel_spmd

N = 8192
NIN = 512
NOUT = 512
NC = 8
RB = N // NC  # 1024 rows per core
KT = N // 128  # 64 k-tiles
K1 = 58  # k-tiles in fp8 DoubleRow; rest in bf16
KB = KT - K1
CH8 = [2, 2, 4] + [8] * 6 + [2]  # fp8-phase chunk sizes (sum = K1, all even)
CHB = [6]  # bf16-phase chunk sizes (sum = KB)
SC = 512.0  # per-operand prescale (e4m3 subnormal floor is 2^-6)
F32 = mybir.dt.float32
BF16 = mybir.dt.bfloat16
FP8 = mybir.dt.float8e4
NPBF = ml_dtypes.bfloat16
NP8 = mybir.dt.np(mybir.dt.float8e4)

_CACHED_NC = None


def _build():
    nc = bacc.Bacc(None, target_bir_lowering=False)

    # Per-core inputs (packed layouts, see kernel() glue)
    S8 = nc.dram_tensor("S8", [128, K1, RB], FP8, kind="ExternalInput")
    Sb = nc.dram_tensor("Sb", [128, KB, RB], BF16, kind="ExternalInput")
    HW8 = nc.dram_tensor("HW8", [128, K1, NOUT], FP8, kind="ExternalInput")
    HWB = nc.dram_tensor("HWB", [128, KB, NOUT], BF16, kind="ExternalInput")
    Bt = nc.dram_tensor("Bt", [128, 4], F32, kind="ExternalInput")
    # Output transposed: col block (c*2+mb)*512 holds OutT[c-chunk, mb-half]
    Out = nc.dram_tensor("out", [128, 8 * 512], BF16, kind="ExternalOutput")

    with tile.TileContext(nc) as tc:
        with (
            tc.tile_pool(name="persist", bufs=1) as persist,
            tc.tile_pool(name="strip8", bufs=6) as strip8p,
            tc.tile_pool(name="stripb", bufs=2) as stripbp,
            tc.tile_pool(name="res", bufs=4) as resp,
            tc.tile_pool(name="acc", bufs=2, space=bass.MemorySpace.PSUM) as pacc,
        ):
            hw8_big = persist.tile([128, K1, NOUT], FP8)
            hwb_big = persist.tile([128, KB, NOUT], BF16)
            b_t = persist.tile([128, 4], F32)

            # Both m-halves accumulate across the whole k loop: 8 banks.
            acc0 = pacc.tile([128, 4 * 512], F32, tag="acc")
            acc1 = pacc.tile([128, 4 * 512], F32, tag="acc")
            accs = (acc0, acc1)

            # HAM warm-up (see module docstring). N=64 keeps each dummy near
            # the NX dispatch floor (~30-55ns) so the warm-up span is
            # insensitive to the HAM clock phase; the bank is overwritten by
            # the real start=True matmul.
            scratch = persist.tile([128, 128], BF16)
            nc.vector.memset(scratch[:], 0.0)
            for _ in range(100):
                nc.tensor.matmul(
                    acc0[:, 0:64], scratch[:], scratch[:, 0:64], start=True, stop=True
                )

            # ---- fp8 DoubleRow phase: kt in [0, K1), 2 k-tiles per matmul
            kt0 = 0
            for ci, cn in enumerate(CH8):
                hsl = slice(kt0, kt0 + cn)
                nc.gpsimd.dma_start(hw8_big[:, hsl, :], HW8[:, hsl, :])
                strip = strip8p.tile([128, 8, RB], FP8, tag="s8")
                for s0 in range(0, cn, 4):
                    sn = min(4, cn - s0)
                    nc.gpsimd.dma_start(
                        strip[:, s0 : s0 + sn, :], S8[:, kt0 + s0 : kt0 + s0 + sn, :]
                    )
                if ci == 1:
                    nc.gpsimd.dma_start(b_t[:], Bt[:, :])
                for ktl in range(0, cn, 2):
                    kt = kt0 + ktl
                    for c in range(4):
                        for mb in range(2):
                            nc.tensor.matmul(
                                accs[mb][:, c * 512 : (c + 1) * 512],
                                hw8_big[:, ktl + kt0 : ktl + kt0 + 2, c * 128 : (c + 1) * 128],
                                strip[:, ktl : ktl + 2, mb * 512 : (mb + 1) * 512],
                                start=(kt == 0),
                                stop=False,
                                perf_mode=mybir.MatmulPerfMode.DoubleRow,
                            )
                kt0 += cn

            # ---- bf16 phase: kt in [K1, KT)
            for ci, cn in enumerate(CHB):
                kb0 = kt0 - K1
                hsl = slice(kb0, kb0 + cn)
                nc.gpsimd.dma_start(hwb_big[:, hsl, :], HWB[:, hsl, :])
                strip = stripbp.tile([128, 8, RB], BF16, tag="sb")
                for s0 in range(0, cn, 4):
                    sn = min(4, cn - s0)
                    nc.gpsimd.dma_start(
                        strip[:, s0 : s0 + sn, :], Sb[:, kb0 + s0 : kb0 + s0 + sn, :]
                    )
                last = ci == len(CHB) - 1
                if not last:
                    for ktl in range(cn):
                        for c in range(4):
                            for mb in range(2):
                                nc.tensor.matmul(
                                    accs[mb][:, c * 512 : (c + 1) * 512],
                                    hwb_big[:, kb0 + ktl, c * 128 : (c + 1) * 128],
                                    strip[:, ktl, mb * 512 : (mb + 1) * 512],
                                    start=False,
                                    stop=False,
                                )
                else:
                    # Bank-major: banks stop staggered; tanh + store overlap
                    # the remaining banks' matmuls.
                    for c in range(4):
                        for mb in range(2):
                            for ktl in range(cn):
                                nc.tensor.matmul(
                                    accs[mb][:, c * 512 : (c + 1) * 512],
                                    hwb_big[:, kb0 + ktl, c * 128 : (c + 1) * 128],
                                    strip[:, ktl, mb * 512 : (mb + 1) * 512],
                                    start=False,
                                    stop=(ktl == cn - 1),
                                )
                            res = resp.tile([128, 512], BF16, tag="res")
                            nc.scalar.activation(
                                res[:],
                                accs[mb][:, c * 512 : (c + 1) * 512],
                                mybir.ActivationFunctionType.Tanh,
                                bias=b_t[:, c : c + 1],
                                scale=float(1.0 / (SC * SC)),
                            )
                            blk = (c * 2 + mb) * 512
                            nc.sync.dma_start(Out[:, blk : blk + 512], res[:])
                kt0 += cn

    nc.compile()
    return nc


def kernel(H, adj_matrix, W, b):
    global _CACHED_NC
    H = np.ascontiguousarray(np.asarray(H, dtype=np.float32))
    adj = np.ascontiguousarray(np.asarray(adj_matrix, dtype=np.float32))
    W = np.asarray(W, dtype=np.float32)
    b = np.asarray(b, dtype=np.float32)

    K1R = K1 * 128  # fp8 k-range in rows

    # Degrees (with self loop), scales
    deg = adj.sum(axis=0, dtype=np.float32) + 1.0
    d = deg.astype(np.float32) ** -0.5
    d = np.where(np.isinf(d), np.float32(0.0), d).astype(np.float32)

    # W folded into H (f32 BLAS), then column scale d and prescale SC
    HsW = (d[:, None] * (H @ W.T)).astype(np.float32)
    HsWs = HsW * np.float32(SC)
    HW8p = (
        HsWs[:K1R].reshape(K1, 128, NOUT).transpose(1, 0, 2)
    ).astype(NP8)
    HWBp = (
        HsWs[K1R:].reshape(KB, 128, NOUT).transpose(1, 0, 2)
    ).astype(NPBF)

    # Per-column mean of S'' over the fp8 k-range (exact, f32):
    # c_m = d_m * (sum_k<K1R adj[m,k] + [m<K1R]) / K1R
    rowsum = adj[:, :K1R].sum(axis=1, dtype=np.float32)
    rowsum[:K1R] += 1.0
    c = (d * rowsum / np.float32(K1R)).astype(np.float32)
    colsum = HsW[:K1R].sum(axis=0, dtype=np.float32)

    # Fold the mean-restore (c_m (x) colsum_n) into the bf16-phase S data:
    # find least-norm w with Hbf_device^T w = colsum (Hbf_device = the
    # quantized bf16 H the device actually contracts with), then add
    # c_m * w_k to the bf16 S block. The bf16 contraction then reproduces
    # the rank-1 correction exactly — no device-side correction matmuls.
    fSC = np.float32(SC)
    Hbf_dev = (HsWs[K1R:].astype(NPBF).astype(np.float32)) / fSC  # [KBR, NOUT]
    w = np.linalg.lstsq(Hbf_dev.T, colsum, rcond=None)[0].astype(np.float32)

    # S''^T via cache-blocked transpose: fp8 rows (centered, x SC) and bf16
    # rows (x SC, with the c (x) w fold), then exact self-loop diagonals.
    adjT8 = np.empty((K1R, N), dtype=NP8)
    adjTb = np.empty((N - K1R, N), dtype=NPBF)
    BLK = 256
    for i in range(0, N, BLK):
        blk = adj[i : i + BLK, :] * d[i : i + BLK, None]  # [m, k] = S''[k,m]^T
        adjT8[:, i : i + BLK] = ((blk[:, :K1R] - c[i : i + BLK, None]) * fSC).T.astype(
            NP8
        )
        adjTb[:, i : i + BLK] = (
            (blk[:, K1R:] + c[i : i + BLK, None] * w[None, :]) * fSC
        ).T.astype(NPBF)
    idx = np.arange(K1R)
    adjT8[idx, idx] = ((d[:K1R] * (adj[idx, idx] + 1.0) - c[:K1R]) * fSC).astype(NP8)
    idxb = np.arange(K1R, N)
    adjTb[idxb - K1R, idxb] = (
        (d[K1R:] * (adj[idxb, idxb] + 1.0) + c[K1R:] * w[idxb - K1R]) * fSC
    ).astype(NPBF)

    Bt = np.ascontiguousarray(b.reshape(4, 128).T)

    in_maps = []
    for cc in range(NC):
        r0, r1 = cc * RB, (cc + 1) * RB
        X8 = adjT8[:, r0:r1].reshape(K1, 128, RB).transpose(1, 0, 2)
        Xb = adjTb[:, r0:r1].reshape(KB, 128, RB).transpose(1, 0, 2)
        in_maps.append(
            {
                "S8": np.ascontiguousarray(X8),
                "Sb": np.ascontiguousarray(Xb),
                "HW8": HW8p,
                "HWB": HWBp,
                "Bt": Bt,
            }
        )

    if _CACHED_NC is None:
        _CACHED_NC = _build()
    globals()["_LAST_IN_MAPS"] = in_maps
    res = run_bass_kernel_spmd(_CACHED_NC, in_maps, core_ids=list(range(NC)))

    out = np.empty((N, NOUT), dtype=np.float32)
    for cc in range(NC):
        r0 = cc * RB
        X = res.results[cc]["out"].reshape(128, 4, 2, 512)
        out[r0 : r0 + RB, :] = (
            X.transpose(2, 3, 1, 0).reshape(RB, NOUT).astype(np.float32)
        )
    return out
